# revision 11
# baseline (speedup 1.0000x reference)
"""Trainium2 Bass kernel: mean over rows of ||A_row - B_row||_2.

Full inputs A, B: [2_000_000, 64] fp32. Data-parallel over 8 NeuronCores:
core c gets rows [c*250_000, (c+1)*250_000). On each core the 250k x 64
block is viewed as [125 partitions, 128_000 floats] (each partition owns
2000 consecutive rows). A and B are interleaved host-side at chunk
granularity into one tensor so each chunk needs a single DMA (the TRN2
TensorTensor ISA slot only fits one semaphore wait, so the subtract must
depend on exactly one DMA).

Per chunk of 100 rows/partition (ab tile = [125, 2*6400]):
    d  = ab[:, :F] - ab[:, F:]   (DVE, in place over the A half)
    d  = d*d                     (ACT, in place)
    rs = rowsum(d)               (DVE reduce innermost 64) -> [125, 100]
    y0 = sqrt(rs)                (ACT; low-precision table)
    u  = y0 + rs/y0              (DVE recip+mul+add) == 2*rownorm + O(eps^2)
    csum[:, k] = sum(u)          (DVE reduce)
Per-core output: per-partition sums [125, 1] of u = 2*rownorm. Host sums
all 8x125 partials in float64 and divides by 2*N.
"""

import sys

import numpy as np

for _p in ("/opt/trn_rl_repo",):
    if _p not in sys.path:
        sys.path.insert(0, _p)

import concourse.bacc as bacc
import concourse.bass as bass
import concourse.mybir as mybir
import concourse.tile as tile
from concourse.bass_utils import run_bass_kernel_spmd

N_ROWS = 2_000_000
D = 64
N_CORES = 8
ROWS_PER_CORE = N_ROWS // N_CORES  # 250_000
P = 125  # SBUF partitions used (250_000 = 125 * 2000)
COLS = ROWS_PER_CORE * D // P  # 128_000 floats per partition
T = 80  # rows per partition per chunk
F = T * D  # 5120 floats per partition per chunk
NCHUNK = COLS // F  # 25

_nc_cache = None
LAST_RESULTS = None  # BassKernelResults of the most recent run (for profiling)


def _build(nchunk=NCHUNK):
    f32 = mybir.dt.float32
    nc = bacc.Bacc(
        "TRN2", target_bir_lowering=False, debug=False, num_devices=N_CORES
    )
    AB = nc.dram_tensor("AB", [P, 2 * COLS], f32, kind="ExternalInput").ap()
    OUT = nc.dram_tensor("OUT", [P, 1], f32, kind="ExternalOutput").ap()

    X = mybir.AxisListType.X
    ADD = mybir.AluOpType.add
    SUB = mybir.AluOpType.subtract
    MUL = mybir.AluOpType.mult

    with tile.TileContext(nc) as tc:
        with (
            tc.tile_pool(name="pab", bufs=3) as pab,
            tc.tile_pool(name="pd", bufs=2) as pd,
            tc.tile_pool(name="small", bufs=2) as ps,
            tc.tile_pool(name="acc", bufs=1) as pacc,
        ):
            csum = pacc.tile([P, nchunk], f32)
            for k in range(nchunk):
                ab = pab.tile([P, 2 * F], f32)
                nc.sync.dma_start(ab[:], AB[:, k * 2 * F : (k + 1) * 2 * F])

                dt = pd.tile([P, F], f32)
                d = dt[:]
                nc.vector.tensor_tensor(d, ab[:, 0:F], ab[:, F : 2 * F], SUB)
                nc.scalar.square(d, d)

                rs = ps.tile([P, T], f32)
                nc.vector.tensor_reduce(
                    rs[:],
                    d.rearrange("p (t e) -> p t e", e=D),
                    axis=X,
                    op=ADD,
                )

                y0 = ps.tile([P, T], f32)
                nc.scalar.sqrt(y0[:], rs[:])
                # One Newton step: u = y0 + rs/y0 = 2*sqrt(rs)*(1 + O(eps^2)).
                # The factor 2 is divided out on the host.
                r = ps.tile([P, T], f32)
                nc.vector.reciprocal(r[:], y0[:])
                t2 = ps.tile([P, T], f32)
                nc.vector.tensor_tensor(t2[:], rs[:], r[:], MUL)
                u = ps.tile([P, T], f32)
                nc.vector.tensor_tensor(u[:], y0[:], t2[:], ADD)

                nc.vector.tensor_reduce(csum[:, k : k + 1], u[:], axis=X, op=ADD)

            tot = pacc.tile([P, 1], f32)
            nc.vector.tensor_reduce(tot[:], csum[:], axis=X, op=ADD)
            nc.sync.dma_start(OUT, tot[:])
    nc.compile()
    return nc


def make_inputs(A, B):
    """[2M, 64] x2 -> {"AB": (cores, 125, 2*COLS)} with A/B interleaved
    at chunk granularity (each chunk is one contiguous DMA)."""
    A8 = np.asarray(A, dtype=np.float32).reshape(N_CORES, P, NCHUNK, F)
    B8 = np.asarray(B, dtype=np.float32).reshape(N_CORES, P, NCHUNK, F)
    AB = np.stack([A8, B8], axis=3)  # (cores, P, NCHUNK, 2, F)
    return {"AB": AB.reshape(N_CORES, P, 2 * COLS)}


def kernel(A, B):
    global _nc_cache, LAST_RESULTS
    ins = make_inputs(A, B)
    if _nc_cache is None:
        _nc_cache = _build()
    nc = _nc_cache
    in_maps = [{k: v[c] for k, v in ins.items()} for c in range(N_CORES)]
    res = run_bass_kernel_spmd(nc, in_maps, core_ids=list(range(N_CORES)))
    LAST_RESULTS = res
    total = 0.0
    for rmap in res.results:
        total += float(np.sum(rmap["OUT"].astype(np.float64)))
    mean = total * 0.5 / N_ROWS
    return np.array(mean, dtype=np.float32)


# revision 12
# speedup vs baseline: 1.0867x; 1.0867x over previous
"""Trainium2 Bass kernel: mean over rows of ||A_row - B_row||_2.

Full inputs A, B: [2_000_000, 64] fp32. Data-parallel over 8 NeuronCores:
core c gets rows [c*250_000, (c+1)*250_000). On each core the 250k x 64
block is viewed as [125 partitions, 128_000 floats] (each partition owns
2000 consecutive rows). A and B are interleaved host-side at chunk
granularity into one tensor so each chunk needs a single DMA (the TRN2
TensorTensor ISA slot only fits one semaphore wait, so the subtract must
depend on exactly one DMA).

Per chunk of 100 rows/partition (ab tile = [125, 2*6400]):
    d  = ab[:, :F] - ab[:, F:]   (DVE, in place over the A half)
    d  = d*d                     (ACT, in place)
    rs = rowsum(d)               (DVE reduce innermost 64) -> [125, 100]
    y0 = sqrt(rs)                (ACT; low-precision table)
    u  = y0 + rs/y0              (DVE recip+mul+add) == 2*rownorm + O(eps^2)
    csum[:, k] = sum(u)          (DVE reduce)
Per-core output: per-partition sums [125, 1] of u = 2*rownorm. Host sums
all 8x125 partials in float64 and divides by 2*N.
"""

import sys

import numpy as np

for _p in ("/opt/trn_rl_repo",):
    if _p not in sys.path:
        sys.path.insert(0, _p)

import concourse.bacc as bacc
import concourse.bass as bass
import concourse.mybir as mybir
import concourse.tile as tile
from concourse.bass_utils import run_bass_kernel_spmd

N_ROWS = 2_000_000
D = 64
N_CORES = 8
ROWS_PER_CORE = N_ROWS // N_CORES  # 250_000
P = 125  # SBUF partitions used (250_000 = 125 * 2000)
COLS = ROWS_PER_CORE * D // P  # 128_000 floats per partition
T = 80  # rows per partition per chunk
F = T * D  # 5120 floats per partition per chunk
NCHUNK = COLS // F  # 25

_nc_cache = None
LAST_RESULTS = None  # BassKernelResults of the most recent run (for profiling)


def _build(nchunk=NCHUNK):
    f32 = mybir.dt.float32
    nc = bacc.Bacc(
        "TRN2", target_bir_lowering=False, debug=False, num_devices=N_CORES
    )
    AB = nc.dram_tensor("AB", [P, 2 * COLS], f32, kind="ExternalInput").ap()
    OUT = nc.dram_tensor("OUT", [P, 1], f32, kind="ExternalOutput").ap()

    X = mybir.AxisListType.X
    ADD = mybir.AluOpType.add
    SUB = mybir.AluOpType.subtract
    MUL = mybir.AluOpType.mult

    with tile.TileContext(nc) as tc:
        with (
            tc.tile_pool(name="pab", bufs=3) as pab,
            tc.tile_pool(name="pd", bufs=2) as pd,
            tc.tile_pool(name="small", bufs=2) as ps,
            tc.tile_pool(name="acc", bufs=1) as pacc,
        ):
            csum = pacc.tile([P, nchunk], f32)
            for k in range(nchunk):
                ab = pab.tile([P, 2 * F], f32)
                # Alternate between the two physical HWDGE rings
                # (qSPDynamicHW via nc.sync, qActDynamicHW via nc.scalar) so
                # successive chunk DMAs issue/track in parallel: 412 -> 379 us
                # in the cost-model timeline.
                dma_eng = nc.scalar if k % 2 else nc.sync
                dma_eng.dma_start(ab[:], AB[:, k * 2 * F : (k + 1) * 2 * F])

                dt = pd.tile([P, F], f32)
                d = dt[:]
                nc.vector.tensor_tensor(d, ab[:, 0:F], ab[:, F : 2 * F], SUB)
                nc.scalar.square(d, d)

                rs = ps.tile([P, T], f32)
                nc.vector.tensor_reduce(
                    rs[:],
                    d.rearrange("p (t e) -> p t e", e=D),
                    axis=X,
                    op=ADD,
                )

                y0 = ps.tile([P, T], f32)
                nc.scalar.sqrt(y0[:], rs[:])
                # One Newton step: u = y0 + rs/y0 = 2*sqrt(rs)*(1 + O(eps^2)).
                # The factor 2 is divided out on the host.
                r = ps.tile([P, T], f32)
                nc.vector.reciprocal(r[:], y0[:])
                t2 = ps.tile([P, T], f32)
                nc.vector.tensor_tensor(t2[:], rs[:], r[:], MUL)
                u = ps.tile([P, T], f32)
                nc.vector.tensor_tensor(u[:], y0[:], t2[:], ADD)

                nc.vector.tensor_reduce(csum[:, k : k + 1], u[:], axis=X, op=ADD)

            tot = pacc.tile([P, 1], f32)
            nc.vector.tensor_reduce(tot[:], csum[:], axis=X, op=ADD)
            nc.sync.dma_start(OUT, tot[:])
    nc.compile()
    return nc


def make_inputs(A, B):
    """[2M, 64] x2 -> {"AB": (cores, 125, 2*COLS)} with A/B interleaved
    at chunk granularity (each chunk is one contiguous DMA)."""
    A8 = np.asarray(A, dtype=np.float32).reshape(N_CORES, P, NCHUNK, F)
    B8 = np.asarray(B, dtype=np.float32).reshape(N_CORES, P, NCHUNK, F)
    AB = np.stack([A8, B8], axis=3)  # (cores, P, NCHUNK, 2, F)
    return {"AB": AB.reshape(N_CORES, P, 2 * COLS)}


def kernel(A, B):
    global _nc_cache, LAST_RESULTS
    ins = make_inputs(A, B)
    if _nc_cache is None:
        _nc_cache = _build()
    nc = _nc_cache
    in_maps = [{k: v[c] for k, v in ins.items()} for c in range(N_CORES)]
    res = run_bass_kernel_spmd(nc, in_maps, core_ids=list(range(N_CORES)))
    LAST_RESULTS = res
    total = 0.0
    for rmap in res.results:
        total += float(np.sum(rmap["OUT"].astype(np.float64)))
    mean = total * 0.5 / N_ROWS
    return np.array(mean, dtype=np.float32)
